# revision 28
# baseline (speedup 1.0000x reference)
"""AttentionAggregator Trainium2 kernel.

B=20000 nodes, K=10 neighbors, N=100000 embed rows, F=256, H=128.
Data-parallel over B across 8 NeuronCores (2500 nodes/core).

Per-core device pipeline (all matmuls bf16 -> fp32 PSUM):
  - transposed gather: dma_gather(transpose=True) from a per-core
    compacted bf16 embedding table (int16 indices) -> X^T in SBUF
    as [128 f-half, 2, rows]
  - T_j^T = tanh(W_ja^T X^T)      (j=1,2,3; two f-half matmuls + ACT tanh)
  - G = (W2b W1b^T) T1^T          (const stationary CmT = W1b W2b^T)
  - per 12-node group g (120 rows): S^T block = T2^T_g.T @ G_g, with a
    rank-13 constant matmul adding -50 off-block (mask)
  - E' = exp(S^T)  (ACT, PSUM->SBUF bf16)
  - with E' as stationary: r = E'^T @ 1 and A = E'^T @ V (V = T3 W3b)
  - As = A * (1/r)  per-query normalize (DVE reciprocal + tensor_scalar)
  - out_g = blockind^T @ As  (sums the K queries per node)
"""

import sys

sys.path.insert(0, "/opt/trn_rl_repo")

import numpy as np
import ml_dtypes

import concourse.bass as bass
import concourse.bacc as bacc
import concourse.mybir as mybir
import concourse.tile as tile
from concourse.bass_utils import run_bass_kernel_spmd


def _patch_ldw_opt():
    """Re-enable walrus LDWEIGHTS optimization (hardcoded off in
    compile_bir_kernel); patches both the source module and bass2jax's
    imported reference."""
    # NOTE: tried forcing walrus --enable-ldw-opt=true; it crashes codegen
    # (visitInstLdweights, CoreV3GenImpl.cpp:694). Keep default.
    return


_patch_ldw_opt()

BF16 = ml_dtypes.bfloat16

B, K, N, F, H = 20000, 10, 100000, 256, 128
NCORES = 8
B_CORE = B // NCORES                # 2500
CHUNK_NODES = 192                   # nodes per chunk
CHUNK_ROWS = CHUNK_NODES * K        # 1920
NCHUNK = 13                         # full chunks; + 1 tail chunk of 48 nodes
TAIL_NODES = 48                     # 1 quad (4 groups of 12)
TAIL_ROWS = 512                     # one gather (48*K=480 used, 512 padded)
PAD_NODES = NCHUNK * CHUNK_NODES + TAIL_NODES   # 2544
PAD_ROWS = NCHUNK * CHUNK_ROWS + TAIL_ROWS      # 25472
IDX_COLS = NCHUNK * (CHUNK_ROWS // 16) + TAIL_ROWS // 16  # 1592
GSIZES = (512, 512, 512, 384)       # per-chunk gather sizes (sum 1920)
GOFFS = (0, 512, 1024, 1536)
GROUP_NODES = 12                    # nodes per attention group
GROUP_ROWS = GROUP_NODES * K        # 120
GROUPS = CHUNK_ROWS // GROUP_ROWS   # 16 per chunk
QUADS = GROUPS // 4                 # 4 (4 groups of 4)
TBL_ROWS = 25600                    # per-core compact table rows (padded)
MASK_L = 50.0

_CACHED = {}


def _build_program():
    nc = bacc.Bacc(
        "TRN2",
        target_bir_lowering=False,
        debug=False,
        num_devices=NCORES,
        num_swdge_queues=4,
    )
    dt = mybir.dt
    f32, bf16, i16 = dt.float32, dt.bfloat16, dt.int16

    table = nc.dram_tensor("table", [TBL_ROWS, F], bf16, kind="ExternalInput")
    idxs = nc.dram_tensor("idxs", [128, IDX_COLS], i16,
                          kind="ExternalInput")
    # packed constants: 8 weight mats | umaskT | vmaskT | blockind | ones
    wnames = ["w1a0", "w1a1", "w2a0", "w2a1", "w3a0", "w3a1", "w3b", "cmt"]
    WPACK = 8 * 128 + 128 + 480 + GROUP_NODES + 1
    wpack = nc.dram_tensor("wpack", [128, WPACK], bf16, kind="ExternalInput")
    out = nc.dram_tensor("out", [PAD_NODES, H], f32, kind="ExternalOutput")

    with tile.TileContext(nc) as tc:
        with (
            tc.tile_pool(name="consts", bufs=1) as cpool,
            tc.tile_pool(name="xt", bufs=2) as xpool,
            tc.tile_pool(name="tp", bufs=2) as tpool,
            tc.tile_pool(name="attn", bufs=2) as apool,
            tc.tile_pool(name="outst", bufs=2) as opool,
            tc.tile_pool(name="ps480", bufs=2, space="PSUM") as ps480p,
            tc.tile_pool(name="pss", bufs=2, space="PSUM") as pssp,
            tc.tile_pool(name="psv", bufs=1, space="PSUM") as psvp,
            tc.tile_pool(name="psr", bufs=1, space="PSUM") as psrp,
            tc.tile_pool(name="psa", bufs=1, space="PSUM") as psap,
            tc.tile_pool(name="pso", bufs=1, space="PSUM") as psop,
        ):
            # ---- load constants to SBUF (single packed DMA) ----
            wp = cpool.tile([128, WPACK], bf16, tag="c_wpack")
            nc.sync.dma_start(out=wp[:, :], in_=wpack[:, :])
            wsb = {n: wp[:, 128 * i:128 * (i + 1)]
                   for i, n in enumerate(wnames)}
            um_sb = wp[0:13, 1024:1152]
            vm_sb = wp[0:13, 1152:1632]
            bi_sb = wp[:, 1632:1632 + GROUP_NODES]
            idx_sb = cpool.tile([128, IDX_COLS], i16,
                                tag="c_idx")
            # chunk-0 indices first so its gathers aren't gated on the rest
            nc.sync.dma_start(out=idx_sb[:, 0:120], in_=idxs[:, 0:120])
            nc.sync.dma_start(out=idx_sb[:, 120:], in_=idxs[:, 120:])

            def quad(t2t, t3t, gt, outst, q, row_base):
                """One quad: 4 groups of 12 nodes starting at row_base."""
                ps_s = pssp.tile([128, 480], f32, tag="pss")
                # mask bias first (start=True over all 480 cols)
                nc.tensor.matmul(ps_s[:, :], um_sb[:, :], vm_sb[:, :],
                                 start=True, stop=False,
                                 skip_group_check=True)
                for qq in range(4):
                    r0 = row_base + GROUP_ROWS * qq
                    nc.tensor.matmul(
                        ps_s[:, 120 * qq:120 * qq + 120],
                        t2t[:, r0:r0 + 128],
                        gt[:, r0:r0 + 120],
                        start=False, stop=True, skip_group_check=True)
                em = apool.tile([128, 488], bf16, tag="em")
                nc.scalar.activation(em[0:120, 0:480], ps_s[0:120, 0:480],
                                     mybir.ActivationFunctionType.Exp)

                # V blocks with a leading ones column per group:
                # v4[p, qq, 0] = 1, v4[p, qq, 1:129] = V_qq
                ps_v = psvp.tile([128, 512], f32, tag="psv")
                for qq in range(4):
                    r0 = row_base + GROUP_ROWS * qq
                    nc.tensor.matmul(ps_v[:, 128 * qq:128 * (qq + 1)],
                                     t3t[:, r0:r0 + 128],
                                     wsb["w3b"][:, :],
                                     start=True, stop=True)
                v4 = apool.tile([128, 4, 129], bf16, tag="v4")
                nc.vector.memset(v4[0:120, :, 0:1], 1.0)
                nc.vector.tensor_copy(
                    v4[0:120, :, 1:129],
                    ps_v[0:120, :].rearrange("p (a b) -> p a b", a=4))

                # merged [r | A] matmuls: lhsT = Em block (one LDW each),
                # rhs = [1 | V] -> psum col 0 = r, cols 1:129 = A
                ras = []
                for h in range(2):
                    ps_ra = psap.tile([128, 2, 129], f32, tag=f"psra{h}")
                    for qh in range(2):
                        qq = 2 * h + qh
                        lhs = em[0:120, 120 * qq:120 * qq + 128]
                        nc.tensor.matmul(ps_ra[:, qh, :], lhs,
                                         v4[0:120, qq, :],
                                         start=True, stop=True)
                    ras.append(ps_ra)
                rec = apool.tile([128, 4], mybir.dt.float32, tag="rec")
                for h in range(2):
                    nc.vector.reciprocal(
                        rec[0:120, 2 * h:2 * h + 2].rearrange(
                            "p (a o) -> p a o", o=1),
                        ras[h][0:120, :, 0:1])
                as4 = apool.tile([128, 512], bf16, tag="as4")
                for qq in range(4):
                    nc.vector.tensor_scalar_mul(
                        as4[0:120, 128 * qq:128 * (qq + 1)],
                        ras[qq // 2][0:120, qq % 2, 1:129],
                        rec[0:120, qq:qq + 1])
                ps_o = psop.tile([128, 512], f32, tag="pso")
                for qq in range(4):
                    nc.tensor.matmul(ps_o[0:GROUP_NODES,
                                          128 * qq:128 * (qq + 1)],
                                     bi_sb[0:120, :],
                                     as4[0:120, 128 * qq:128 * (qq + 1)],
                                     start=True, stop=True)
                nc.vector.tensor_copy(
                    outst[0:GROUP_NODES, 512 * q:512 * (q + 1)],
                    ps_o[0:GROUP_NODES, 0:512])

            for c in range(NCHUNK):
                # ---- gather chunk: X^T layout tiles ----
                # chunk 0 uses small gathers so the PE isn't gated on the
                # serial Q7 descriptor-generation latency at kernel start
                gsizes = (256,) * 7 + (128,) if c == 0 else GSIZES
                goffs = [sum(gsizes[:i]) for i in range(len(gsizes))]
                xts = []
                for gi, gsz in enumerate(gsizes):
                    xt = xpool.tile([128, 2, gsz], bf16, tag=f"xt{gi}")
                    nc.gpsimd.dma_gather(
                        out_ap=xt[:, :, :],
                        in_ap=table[:, :],
                        idxs_ap=idx_sb[:, c * 120 + goffs[gi] // 16:
                                       c * 120 + (goffs[gi] + gsz) // 16],
                        num_idxs=gsz,
                        num_idxs_reg=gsz,
                        elem_size=F,
                        transpose=True,
                        queue_num=gi % 4,
                    )
                    xts.append(xt)

                # ---- first layers: T_j^T = tanh(W_ja^T X^T), G ----
                t1 = tpool.tile([128, CHUNK_ROWS + 8], bf16, tag="t1")
                t2 = tpool.tile([128, CHUNK_ROWS + 8], bf16, tag="t2")
                t3 = tpool.tile([128, CHUNK_ROWS + 8], bf16, tag="t3")
                g = tpool.tile([128, CHUNK_ROWS], bf16, tag="g")
                for gi, gsz in enumerate(gsizes):
                    xt = xts[gi]
                    csl = slice(goffs[gi], goffs[gi] + gsz)
                    for w0, w1, tj in (("w1a0", "w1a1", t1),
                                       ("w2a0", "w2a1", t2),
                                       ("w3a0", "w3a1", t3)):
                        ps = ps480p.tile([128, 512], f32, tag="ps480")
                        nc.tensor.matmul(ps[:, 0:gsz], wsb[w0][:, :],
                                         xt[:, 0, :],
                                         start=True, stop=False)
                        nc.tensor.matmul(ps[:, 0:gsz], wsb[w1][:, :],
                                         xt[:, 1, :],
                                         start=False, stop=True)
                        nc.scalar.activation(
                            tj[:, csl], ps[:, 0:gsz],
                            mybir.ActivationFunctionType.Tanh)
                    psg = ps480p.tile([128, 512], f32, tag="ps480")
                    nc.tensor.matmul(psg[:, 0:gsz], wsb["cmt"][:, :],
                                     t1[:, csl], start=True, stop=True)
                    nc.scalar.activation(
                        g[:, csl], psg[:, 0:gsz],
                        mybir.ActivationFunctionType.Copy)

                # ---- attention: 4 quads of 4 groups ----
                outst = opool.tile([GROUP_NODES, GROUPS * H], f32, tag="outst")
                for q in range(QUADS):
                    quad(t2, t3, g, outst, q, 480 * q)

                # ---- store chunk output ----
                dst = out[c * CHUNK_NODES:(c + 1) * CHUNK_NODES, :].rearrange(
                    "(b i) d -> i b d", i=GROUP_NODES)
                src = outst[0:GROUP_NODES, :].rearrange(
                    "i (b d) -> i b d", b=GROUPS)
                nc.sync.dma_start(out=dst, in_=src)

            # ---- tail chunk: 48 real+pad nodes (1 gather, 1 quad) ----
            xt = xpool.tile([128, 2, 512], bf16, tag="xt0")
            nc.gpsimd.dma_gather(
                out_ap=xt[:, :, :], in_ap=table[:, :],
                idxs_ap=idx_sb[:, NCHUNK * 120:NCHUNK * 120 + 32],
                num_idxs=512, num_idxs_reg=512, elem_size=F,
                transpose=True, queue_num=3,
            )
            tt1 = tpool.tile([128, 520], bf16, tag="tt1")
            tt2 = tpool.tile([128, 520], bf16, tag="tt2")
            tt3 = tpool.tile([128, 520], bf16, tag="tt3")
            tg = tpool.tile([128, 512], bf16, tag="tg")
            for w0, w1, tj in (("w1a0", "w1a1", tt1),
                               ("w2a0", "w2a1", tt2),
                               ("w3a0", "w3a1", tt3)):
                ps = ps480p.tile([128, 512], f32, tag="ps480")
                nc.tensor.matmul(ps[:, :], wsb[w0][:, :], xt[:, 0, :],
                                 start=True, stop=False)
                nc.tensor.matmul(ps[:, :], wsb[w1][:, :], xt[:, 1, :],
                                 start=False, stop=True)
                nc.scalar.activation(tj[:, 0:512], ps[:, :],
                                     mybir.ActivationFunctionType.Tanh)
            psg = ps480p.tile([128, 512], f32, tag="ps480")
            nc.tensor.matmul(psg[:, :], wsb["cmt"][:, :], tt1[:, 0:512],
                             start=True, stop=True)
            nc.scalar.activation(tg[:, :], psg[:, :],
                                 mybir.ActivationFunctionType.Copy)
            toutst = opool.tile([GROUP_NODES, 512], f32, tag="toutst")
            quad(tt2, tt3, tg, toutst, 0, 0)
            dst = out[NCHUNK * CHUNK_NODES:PAD_NODES, :].rearrange(
                "(b i) d -> i b d", i=GROUP_NODES)
            src = toutst[0:GROUP_NODES, 0:512].rearrange(
                "i (b d) -> i b d", b=4)
            nc.sync.dma_start(out=dst, in_=src)

    nc.finalize()
    return nc


def _host_prep(neighbors, embed_table, W1a, W1b, W2a, W2b, W3a, W3b):
    """Shard + build per-core input maps."""
    embed_table = np.asarray(embed_table)
    ebf = np.ascontiguousarray(embed_table.astype(BF16))

    def b(x):
        return np.ascontiguousarray(np.asarray(x).astype(BF16))

    w1a, w2a, w3a = (np.asarray(w, np.float32) for w in (W1a, W2a, W3a))
    wmats = [
        w1a[0:128], w1a[128:256], w2a[0:128], w2a[128:256],
        w3a[0:128], w3a[128:256], np.asarray(W3b, np.float32),
        np.asarray(W1b, np.float32) @ np.asarray(W2b, np.float32).T,
    ]
    # mask = U @ Vm^T adds 0 in-block, -L off-block (rank 13)
    bi = np.zeros((120, GROUP_NODES), np.float32)
    for p in range(120):
        bi[p, p // K] = 1.0
    um = np.zeros((128, 128), np.float32)
    um[0:12, 0:120] = bi.T
    um[12, 0:120] = 1.0
    vm = np.zeros((128, 480), np.float32)
    for qq in range(4):
        vm[0:12, 120 * qq:120 * (qq + 1)] = MASK_L * bi.T
        vm[12, 120 * qq:120 * (qq + 1)] = -MASK_L
    bi128 = np.zeros((128, GROUP_NODES), np.float32)
    bi128[0:120] = bi
    wpack = np.concatenate(
        wmats + [um, vm, bi128, np.ones((128, 1), np.float32)], axis=1)
    shared = {"wpack": b(wpack)}

    nbr = np.asarray(neighbors).astype(np.int64)
    in_maps = []
    for c in range(NCORES):
        nb_c = nbr[c * B_CORE:(c + 1) * B_CORE]           # [2500, 10]
        uniq, inv = np.unique(nb_c, return_inverse=True)
        assert uniq.size <= TBL_ROWS
        tbl = np.zeros((TBL_ROWS, F), BF16)
        tbl[:uniq.size] = ebf[uniq]
        flat = np.zeros(PAD_ROWS, np.int16)
        flat[:nb_c.size] = inv.astype(np.int16).ravel()
        # wrap: index j of a chunk at [j % 16, j // 16], replicated to 128
        idx128 = np.zeros((128, IDX_COLS), np.int16)
        col = row = 0
        for sz in [CHUNK_ROWS] * NCHUNK + [TAIL_ROWS]:
            blk = flat[row:row + sz].reshape(sz // 16, 16).T
            idx128[:, col:col + sz // 16] = np.tile(blk, (8, 1))
            row += sz
            col += sz // 16
        in_maps.append({
            "table": tbl,
            "idxs": idx128,
            **{k: v for k, v in shared.items()},
        })
    return in_maps


def kernel(neighbors, embed_table, W1a, W1b, W2a, W2b, W3a, W3b, _trace=False,
           **trace_kwargs):
    key = "prog"
    if key not in _CACHED:
        _CACHED[key] = _build_program()
    nc = _CACHED[key]
    in_maps = _host_prep(neighbors, embed_table, W1a, W1b, W2a, W2b, W3a, W3b)
    res = run_bass_kernel_spmd(nc, in_maps, list(range(NCORES)),
                               trace=_trace, **trace_kwargs)
    outs = [res.results[c]["out"][:B_CORE] for c in range(NCORES)]
    full = np.concatenate(outs, axis=0).astype(np.float32)
    kernel.last_results = res
    return full


# revision 34
# speedup vs baseline: 1.0208x; 1.0208x over previous
"""AttentionAggregator Trainium2 kernel.

B=20000 nodes, K=10 neighbors, N=100000 embed rows, F=256, H=128.
Data-parallel over B across 8 NeuronCores (2500 nodes/core).

Per-core device pipeline (all matmuls bf16 -> fp32 PSUM):
  - transposed gather: dma_gather(transpose=True) from a per-core
    compacted bf16 embedding table (int16 indices) -> X^T in SBUF
    as [128 f-half, 2, rows]
  - T_j^T = tanh(W_ja^T X^T)      (j=1,2,3; two f-half matmuls + ACT tanh)
  - G = (W2b W1b^T) T1^T          (const stationary CmT = W1b W2b^T)
  - per 12-node group g (120 rows): S^T block = T2^T_g.T @ G_g, with a
    rank-13 constant matmul adding -50 off-block (mask)
  - E' = exp(S^T)  (ACT, PSUM->SBUF bf16)
  - with E' as stationary: r = E'^T @ 1 and A = E'^T @ V (V = T3 W3b)
  - As = A * (1/r)  per-query normalize (DVE reciprocal + tensor_scalar)
  - out_g = blockind^T @ As  (sums the K queries per node)
"""

import sys

sys.path.insert(0, "/opt/trn_rl_repo")

import numpy as np
import ml_dtypes

import concourse.bass as bass
import concourse.bacc as bacc
import concourse.mybir as mybir
import concourse.tile as tile
from concourse.bass_utils import run_bass_kernel_spmd


def _patch_ldw_opt():
    """Re-enable walrus LDWEIGHTS optimization (hardcoded off in
    compile_bir_kernel); patches both the source module and bass2jax's
    imported reference."""
    # NOTE: tried forcing walrus --enable-ldw-opt=true; it crashes codegen
    # (visitInstLdweights, CoreV3GenImpl.cpp:694). Keep default.
    return


_patch_ldw_opt()

BF16 = ml_dtypes.bfloat16

B, K, N, F, H = 20000, 10, 100000, 256, 128
NCORES = 8
B_CORE = B // NCORES                # 2500
CHUNK_NODES = 192                   # nodes per chunk
CHUNK_ROWS = CHUNK_NODES * K        # 1920
NCHUNK = 13                         # full chunks; + 1 tail chunk of 48 nodes
TAIL_NODES = 48                     # 1 quad (4 groups of 12)
TAIL_ROWS = 512                     # one gather (48*K=480 used, 512 padded)
PAD_NODES = NCHUNK * CHUNK_NODES + TAIL_NODES   # 2544
PAD_ROWS = NCHUNK * CHUNK_ROWS + TAIL_ROWS      # 25472
IDX_COLS = NCHUNK * (CHUNK_ROWS // 16) + TAIL_ROWS // 16  # 1592
GSIZES = (512, 512, 512, 384)       # per-chunk gather sizes (sum 1920)
GOFFS = (0, 512, 1024, 1536)
GROUP_NODES = 12                    # nodes per attention group
GROUP_ROWS = GROUP_NODES * K        # 120
GROUPS = CHUNK_ROWS // GROUP_ROWS   # 16 per chunk
QUADS = GROUPS // 4                 # 4 (4 groups of 4)
TBL_ROWS = 25600                    # per-core compact table rows (padded)
MASK_L = 50.0

_CACHED = {}


def _build_program():
    nc = bacc.Bacc(
        "TRN2",
        target_bir_lowering=False,
        debug=False,
        num_devices=NCORES,
        num_swdge_queues=4,
    )
    dt = mybir.dt
    f32, bf16, i16 = dt.float32, dt.bfloat16, dt.int16

    table = nc.dram_tensor("table", [TBL_ROWS, F], bf16, kind="ExternalInput")
    idxs = nc.dram_tensor("idxs", [128, IDX_COLS], i16,
                          kind="ExternalInput")
    # packed constants: 8 weight mats | umaskT | vmaskT | blockind | ones
    wnames = ["w1a0", "w1a1", "w2a0", "w2a1", "w3a0", "w3a1", "w3b", "cmt"]
    WPACK = 8 * 128 + 128 + 480 + GROUP_NODES + 1
    wpack = nc.dram_tensor("wpack", [128, WPACK], bf16, kind="ExternalInput")
    out = nc.dram_tensor("out", [PAD_NODES, H], f32, kind="ExternalOutput")

    with tile.TileContext(nc) as tc:
        with (
            tc.tile_pool(name="consts", bufs=1) as cpool,
            tc.tile_pool(name="xt", bufs=2) as xpool,
            tc.tile_pool(name="tp", bufs=2) as tpool,
            tc.tile_pool(name="attn", bufs=2) as apool,
            tc.tile_pool(name="outst", bufs=2) as opool,
            tc.tile_pool(name="ps480", bufs=2, space="PSUM") as ps480p,
            tc.tile_pool(name="pss", bufs=2, space="PSUM") as pssp,
            tc.tile_pool(name="psv", bufs=1, space="PSUM") as psvp,
            tc.tile_pool(name="psr", bufs=1, space="PSUM") as psrp,
            tc.tile_pool(name="psa", bufs=1, space="PSUM") as psap,
            tc.tile_pool(name="pso", bufs=1, space="PSUM") as psop,
        ):
            # ---- load constants to SBUF (single packed DMA) ----
            wp = cpool.tile([128, WPACK], bf16, tag="c_wpack")
            nc.sync.dma_start(out=wp[:, :], in_=wpack[:, :])
            wsb = {n: wp[:, 128 * i:128 * (i + 1)]
                   for i, n in enumerate(wnames)}
            um_sb = wp[0:13, 1024:1152]
            vm_sb = wp[0:13, 1152:1632]
            bi_sb = wp[:, 1632:1632 + GROUP_NODES]
            idx_sb = cpool.tile([128, IDX_COLS], i16,
                                tag="c_idx")
            # chunk-0 indices first so its gathers aren't gated on the rest
            nc.sync.dma_start(out=idx_sb[:, 0:120], in_=idxs[:, 0:120])
            nc.sync.dma_start(out=idx_sb[:, 120:], in_=idxs[:, 120:])

            def quad(t2t, t3t, gt, outst, q, row_base):
                """One quad: 4 groups of 12 nodes starting at row_base."""
                ps_s = pssp.tile([128, 480], f32, tag="pss")
                # mask bias first (start=True over all 480 cols)
                nc.tensor.matmul(ps_s[:, :], um_sb[:, :], vm_sb[:, :],
                                 start=True, stop=False,
                                 skip_group_check=True)
                for qq in range(4):
                    r0 = row_base + GROUP_ROWS * qq
                    nc.tensor.matmul(
                        ps_s[:, 120 * qq:120 * qq + 120],
                        t2t[:, r0:r0 + 128],
                        gt[:, r0:r0 + 120],
                        start=False, stop=True, skip_group_check=True)
                em = apool.tile([128, 488], bf16, tag="em")
                nc.scalar.activation(em[0:120, 0:480], ps_s[0:120, 0:480],
                                     mybir.ActivationFunctionType.Exp)

                # V blocks with a leading ones column per group:
                # v4[p, qq, 0] = 1, v4[p, qq, 1:129] = V_qq
                ps_v = psvp.tile([128, 512], f32, tag="psv")
                for qq in range(4):
                    r0 = row_base + GROUP_ROWS * qq
                    nc.tensor.matmul(ps_v[:, 128 * qq:128 * (qq + 1)],
                                     t3t[:, r0:r0 + 128],
                                     wsb["w3b"][:, :],
                                     start=True, stop=True)
                v4 = apool.tile([128, 4, 129], bf16, tag="v4")
                nc.vector.memset(v4[0:120, :, 0:1], 1.0)
                nc.vector.tensor_copy(
                    v4[0:120, :, 1:129],
                    ps_v[0:120, :].rearrange("p (a b) -> p a b", a=4))

                # merged [r | A] matmuls: lhsT = Em block (one LDW each),
                # rhs = [1 | V] -> psum col 0 = r, cols 1:129 = A
                ras = []
                for h in range(2):
                    ps_ra = psap.tile([128, 2, 129], f32, tag=f"psra{h}")
                    for qh in range(2):
                        qq = 2 * h + qh
                        lhs = em[0:120, 120 * qq:120 * qq + 128]
                        nc.tensor.matmul(ps_ra[:, qh, :], lhs,
                                         v4[0:120, qq, :],
                                         start=True, stop=True)
                    ras.append(ps_ra)
                rec = apool.tile([128, 4], mybir.dt.float32, tag="rec")
                for h in range(2):
                    nc.vector.reciprocal(
                        rec[0:120, 2 * h:2 * h + 2].rearrange(
                            "p (a o) -> p a o", o=1),
                        ras[h][0:120, :, 0:1])
                as4 = apool.tile([128, 512], bf16, tag="as4")
                for qq in range(4):
                    nc.vector.tensor_scalar_mul(
                        as4[0:120, 128 * qq:128 * (qq + 1)],
                        ras[qq // 2][0:120, qq % 2, 1:129],
                        rec[0:120, qq:qq + 1])
                # 4 out-matmuls packed into the PE's 32-col groups: they run
                # concurrently, each writing 12 rows at psum base 32*qq
                ps_o = psop.tile([128, 128], f32, tag="pso")
                for qq in range(4):
                    nc.tensor.matmul(ps_o[32 * qq:32 * qq + GROUP_NODES, :],
                                     bi_sb[0:120, :],
                                     as4[0:120, 128 * qq:128 * (qq + 1)],
                                     start=True, stop=True,
                                     tile_position=(0, 32 * qq))
                nc.vector.tensor_copy(
                    outst[:, 128 * q:128 * (q + 1)], ps_o[:, :])

            for c in range(NCHUNK):
                # ---- gather chunk: 512/512/512/384 rows, X^T layout ----
                xts = []
                for gi in range(4):
                    gsz = GSIZES[gi]
                    xt = xpool.tile([128, 2, gsz], bf16, tag=f"xt{gi}")
                    nc.gpsimd.dma_gather(
                        out_ap=xt[:, :, :],
                        in_ap=table[:, :],
                        idxs_ap=idx_sb[:, c * 120 + GOFFS[gi] // 16:
                                       c * 120 + (GOFFS[gi] + gsz) // 16],
                        num_idxs=gsz,
                        num_idxs_reg=gsz,
                        elem_size=F,
                        transpose=True,
                        queue_num=(4 * c + gi) % 4,
                    )
                    xts.append(xt)

                # ---- first layers: T_j^T = tanh(W_ja^T X^T), G ----
                t1 = tpool.tile([128, CHUNK_ROWS + 8], bf16, tag="t1")
                t2 = tpool.tile([128, CHUNK_ROWS + 8], bf16, tag="t2")
                t3 = tpool.tile([128, CHUNK_ROWS + 8], bf16, tag="t3")
                g = tpool.tile([128, CHUNK_ROWS], bf16, tag="g")
                for gi in range(4):
                    xt = xts[gi]
                    gsz = GSIZES[gi]
                    csl = slice(GOFFS[gi], GOFFS[gi] + gsz)
                    for w0, w1, tj in (("w1a0", "w1a1", t1),
                                       ("w2a0", "w2a1", t2),
                                       ("w3a0", "w3a1", t3)):
                        ps = ps480p.tile([128, 512], f32, tag="ps480")
                        nc.tensor.matmul(ps[:, 0:gsz], wsb[w0][:, :],
                                         xt[:, 0, :],
                                         start=True, stop=False)
                        nc.tensor.matmul(ps[:, 0:gsz], wsb[w1][:, :],
                                         xt[:, 1, :],
                                         start=False, stop=True)
                        nc.scalar.activation(
                            tj[:, csl], ps[:, 0:gsz],
                            mybir.ActivationFunctionType.Tanh)
                    psg = ps480p.tile([128, 512], f32, tag="ps480")
                    nc.tensor.matmul(psg[:, 0:gsz], wsb["cmt"][:, :],
                                     t1[:, csl], start=True, stop=True)
                    nc.scalar.activation(
                        g[:, csl], psg[:, 0:gsz],
                        mybir.ActivationFunctionType.Copy)

                # ---- attention: 4 quads of 4 groups ----
                # node layout: slab qq holds partitions 32qq..32qq+12, one
                # 128-col block per quad; host permutes nodes so slab qq maps
                # to contiguous out rows [48qq, 48(qq+1)) of the chunk
                outst = opool.tile([128, QUADS * H], f32, tag="outst")
                for q in range(QUADS):
                    quad(t2, t3, g, outst, q, 480 * q)

                # ---- store chunk output: one DMA per 12-row slab ----
                for qq in range(4):
                    dst = out[c * CHUNK_NODES + 48 * qq:
                              c * CHUNK_NODES + 48 * (qq + 1), :].rearrange(
                        "(q i) d -> i q d", i=GROUP_NODES)
                    src = outst[32 * qq:32 * qq + GROUP_NODES, :].rearrange(
                        "p (q d) -> p q d", q=QUADS)
                    nc.sync.dma_start(out=dst, in_=src)

            # ---- tail chunk: 48 real+pad nodes (1 gather, 1 quad) ----
            xt = xpool.tile([128, 2, 512], bf16, tag="xt0")
            nc.gpsimd.dma_gather(
                out_ap=xt[:, :, :], in_ap=table[:, :],
                idxs_ap=idx_sb[:, NCHUNK * 120:NCHUNK * 120 + 32],
                num_idxs=512, num_idxs_reg=512, elem_size=F,
                transpose=True, queue_num=3,
            )
            tt1 = tpool.tile([128, 520], bf16, tag="tt1")
            tt2 = tpool.tile([128, 520], bf16, tag="tt2")
            tt3 = tpool.tile([128, 520], bf16, tag="tt3")
            tg = tpool.tile([128, 512], bf16, tag="tg")
            for w0, w1, tj in (("w1a0", "w1a1", tt1),
                               ("w2a0", "w2a1", tt2),
                               ("w3a0", "w3a1", tt3)):
                ps = ps480p.tile([128, 512], f32, tag="ps480")
                nc.tensor.matmul(ps[:, :], wsb[w0][:, :], xt[:, 0, :],
                                 start=True, stop=False)
                nc.tensor.matmul(ps[:, :], wsb[w1][:, :], xt[:, 1, :],
                                 start=False, stop=True)
                nc.scalar.activation(tj[:, 0:512], ps[:, :],
                                     mybir.ActivationFunctionType.Tanh)
            psg = ps480p.tile([128, 512], f32, tag="ps480")
            nc.tensor.matmul(psg[:, :], wsb["cmt"][:, :], tt1[:, 0:512],
                             start=True, stop=True)
            nc.scalar.activation(tg[:, :], psg[:, :],
                                 mybir.ActivationFunctionType.Copy)
            toutst = opool.tile([128, H], f32, tag="toutst")
            quad(tt2, tt3, tg, toutst, 0, 0)
            for qq in range(4):
                dst = out[NCHUNK * CHUNK_NODES + GROUP_NODES * qq:
                          NCHUNK * CHUNK_NODES + GROUP_NODES * (qq + 1), :]
                nc.sync.dma_start(
                    out=dst,
                    in_=toutst[32 * qq:32 * qq + GROUP_NODES, 0:H])

    nc.finalize()
    return nc


def _host_prep(neighbors, embed_table, W1a, W1b, W2a, W2b, W3a, W3b):
    """Shard + build per-core input maps."""
    embed_table = np.asarray(embed_table)
    ebf = np.ascontiguousarray(embed_table.astype(BF16))

    def b(x):
        return np.ascontiguousarray(np.asarray(x).astype(BF16))

    w1a, w2a, w3a = (np.asarray(w, np.float32) for w in (W1a, W2a, W3a))
    wmats = [
        w1a[0:128], w1a[128:256], w2a[0:128], w2a[128:256],
        w3a[0:128], w3a[128:256], np.asarray(W3b, np.float32),
        np.asarray(W1b, np.float32) @ np.asarray(W2b, np.float32).T,
    ]
    # mask = U @ Vm^T adds 0 in-block, -L off-block (rank 13)
    bi = np.zeros((120, GROUP_NODES), np.float32)
    for p in range(120):
        bi[p, p // K] = 1.0
    um = np.zeros((128, 128), np.float32)
    um[0:12, 0:120] = bi.T
    um[12, 0:120] = 1.0
    vm = np.zeros((128, 480), np.float32)
    for qq in range(4):
        vm[0:12, 120 * qq:120 * (qq + 1)] = MASK_L * bi.T
        vm[12, 120 * qq:120 * (qq + 1)] = -MASK_L
    bi128 = np.zeros((128, GROUP_NODES), np.float32)
    bi128[0:120] = bi
    wpack = np.concatenate(
        wmats + [um, vm, bi128, np.ones((128, 1), np.float32)], axis=1)
    shared = {"wpack": b(wpack)}

    nbr = np.asarray(neighbors).astype(np.int64)
    in_maps = []
    for c in range(NCORES):
        nb_c = nbr[c * B_CORE:(c + 1) * B_CORE]           # [2500, 10]
        uniq, inv = np.unique(nb_c, return_inverse=True)
        assert uniq.size <= TBL_ROWS
        tbl = np.zeros((TBL_ROWS, F), BF16)
        tbl[:uniq.size] = ebf[uniq]
        # permute nodes within full chunks so the packed-psum output slabs
        # land on contiguous out rows: slot 12*(4q+qq)+j <- node 48qq+12q+j
        perm = np.empty(CHUNK_NODES, np.int64)
        for pq in range(4):
            for pqq in range(4):
                for pj in range(GROUP_NODES):
                    perm[GROUP_NODES * (4 * pq + pqq) + pj] = \
                        48 * pqq + GROUP_NODES * pq + pj
        nodes = np.zeros((PAD_NODES, K), np.int16)
        nodes[:B_CORE] = inv.astype(np.int16).reshape(B_CORE, K)
        for ch in range(NCHUNK):
            blk = nodes[ch * CHUNK_NODES:(ch + 1) * CHUNK_NODES].copy()
            nodes[ch * CHUNK_NODES:(ch + 1) * CHUNK_NODES] = blk[perm]
        flat = np.zeros(PAD_ROWS, np.int16)
        flat[:nodes.size] = nodes.ravel()
        # wrap: index j of a chunk at [j % 16, j // 16], replicated to 128
        idx128 = np.zeros((128, IDX_COLS), np.int16)
        col = row = 0
        for sz in [CHUNK_ROWS] * NCHUNK + [TAIL_ROWS]:
            blk = flat[row:row + sz].reshape(sz // 16, 16).T
            idx128[:, col:col + sz // 16] = np.tile(blk, (8, 1))
            row += sz
            col += sz // 16
        in_maps.append({
            "table": tbl,
            "idxs": idx128,
            **{k: v for k, v in shared.items()},
        })
    return in_maps


def kernel(neighbors, embed_table, W1a, W1b, W2a, W2b, W3a, W3b, _trace=False,
           **trace_kwargs):
    key = "prog"
    if key not in _CACHED:
        _CACHED[key] = _build_program()
    nc = _CACHED[key]
    in_maps = _host_prep(neighbors, embed_table, W1a, W1b, W2a, W2b, W3a, W3b)
    res = run_bass_kernel_spmd(nc, in_maps, list(range(NCORES)),
                               trace=_trace, **trace_kwargs)
    outs = [res.results[c]["out"][:B_CORE] for c in range(NCORES)]
    full = np.concatenate(outs, axis=0).astype(np.float32)
    kernel.last_results = res
    return full
